# revision 37
# baseline (speedup 1.0000x reference)
"""DiffTransformerBlock on 8 Trainium2 NeuronCores (Bass/Tile) — v4.

Sharding: core c -> (batch b = c//4, head-group g = c%4; 4 heads each).
Activations kept transposed (feature, seq).  Attention output is
ReduceScattered over sequence quarters (residual folded into the
payload), then attn-norm + SwiGLU (full hidden dim, weights streamed)
run per s-quarter locally.  v2 baseline: 737us HW / 722us sim.
This version: 639us HW / 533us sim, rel err 1.59e-2.

Numerics (max-rel-err budget 2e-2 is outlier-driven; matmuls do NOT
average away independent per-element input noise, so fp8's ~3.6%
element noise passes through GEMMs undamped):
 - fp8e4m3+DoubleRow only on noise-tolerant paths: exp scores (es),
   values (vt) — softmax-weighted averaging; un8 @ wo8 — the Wo output
   is ~0.13 std against the unit residual.  Weights there are scaled
   x32 host-side to center the fp8 range; descales fold into evac /
   STT scalars.
 - fp16 (1 cyc/row like bf16, 10-bit mantissa) for the QKV
   projections, scores q/k, and the whole SwiGLU.
   (A split-fp8 hi+lo 3-term DoubleRow variant is 25% cheaper in the
   cost model but measured +150us SLOWER on real HW — per-call
   LDWEIGHTS between alternating hi/lo stationaries is not free as
   modeled.  Reverted.)

Schedule/engines:
 - FFN weights streamed as 256-col fp16 slabs with >=512B DMA lines
   (v2 streamed 256B lines at half DMA rate: 303us of DMA), slab
   prefetch into the idle attention-phase DMA windows.
 - subln rstd batched at heads-end: all Lns then all Exps — exactly 2
   ACT LUT loads (interleaving Ln/Exp thrashed natural_log <-> exp
   sets, 12 x 1.28us in v2).
 - Heads-phase PSUM: scores double-buffered (4 banks) + per-c U
   accumulation (2) + rowsum banks r0/r1 (2, ss reuses r0 after the
   ratio is consumed).  PE/DVE writes must start at partition 0
   (walrus rejects 32/64/96 quadrant offsets for matmul dst).
 - EV/combine work of head h-1 interleaved into head h's scores
   emission (in-order engines: explicit issue interleave is the only
   way to fill PE stalls while ACT exp is the heads-phase floor).
 - scalar_tensor_tensor fusions: Wo evac + residual add (one DVE op),
   norm applications (x * normw[p] * rstd_bcast), squares on DVE.
 - Single ReduceScatter (ECH=1): the collective cost model has a 15us
   fixed overhead per call; one 4MiB RS beats two 2MiB ones.
 - Pool lifetimes follow the tile allocator's stack discipline; pools
   are scoped to phases so the projection weights, FFN slabs, and
   attention activations time-share SBUF (~208KB/partition budget).
"""

import sys
import contextlib
from collections import deque

if '/opt/trn_rl_repo' not in sys.path:
    sys.path.insert(0, '/opt/trn_rl_repo')

import numpy as np
import ml_dtypes

import concourse.bass as bass  # noqa: F401
import concourse.tile as tile
import concourse.mybir as mybir
from concourse import bacc
from concourse.bass_utils import run_bass_kernel_spmd

F32 = mybir.dt.float32
BF16 = mybir.dt.bfloat16
F16 = mybir.dt.float16
F8E4 = mybir.dt.float8e4
PM_DR = mybir.MatmulPerfMode.DoubleRow
AF = mybir.ActivationFunctionType
ALU = mybir.AluOpType

B, S, E = 2, 1024, 2048
H, D = 16, 128
HG = 4                      # heads per core
HF = HG * 2 * D             # 1024: per-core q/k feature slice
VD = HG * 2 * D             # 1024: per-core v feature slice
HID = 2 * E                 # 4096
LAMBDA_INIT = 0.8
EPS = 1e-6
SCALE = 1.0 / float(np.sqrt(D))
N_CORES = 8
GROUPS = [[0, 1, 2, 3], [4, 5, 6, 7]]
WS = 32.0                   # host-side fp8 weight scale

EB = E // 128               # 16 e-blocks
KB = EB                     # contraction blocks over E
FB = HF // 128              # 8 q/k/v feature blocks
TB = S // 128               # 8 t-blocks
SQ = S // 4                 # 256: per-core sequence quarter
MB = HID // 128             # 32 hidden blocks (full hidden, s-sharded)
ECH = 1                     # attention RS chunks (each E/ECH rows)
EMB = EB // ECH             # m-blocks per RS chunk
HSL = 16                    # w1/w3 hidden slabs (256 fp16 cols each)
ESL = 8                     # w2 e slabs (256 fp16 cols each)


def _build_nc(reps=1):
    nc = bacc.Bacc("TRN2", target_bir_lowering=False, debug=False,
                   num_devices=N_CORES)

    xt_d = nc.dram_tensor("xt", [E, S], F16, kind="ExternalInput")
    wq_d = nc.dram_tensor("wq", [E, HF], F16, kind="ExternalInput")
    wk_d = nc.dram_tensor("wk", [E, HF], F16, kind="ExternalInput")
    wv_d = nc.dram_tensor("wv", [E, VD], F16, kind="ExternalInput")
    wo_d = nc.dram_tensor("wo", [HF, E], F8E4, kind="ExternalInput")
    w1_d = nc.dram_tensor("w1", [E, HID], F16, kind="ExternalInput")
    w3_d = nc.dram_tensor("w3", [E, HID], F16, kind="ExternalInput")
    w2_d = nc.dram_tensor("w2", [HID, E], F16, kind="ExternalInput")
    normw_d = nc.dram_tensor("normw", [128, 2 * EB], F32,
                             kind="ExternalInput")
    lam_d = nc.dram_tensor("lam", [1, 1], F32, kind="ExternalInput")
    out_d = nc.dram_tensor("out", [E, SQ], F32, kind="ExternalOutput")

    xt_r = xt_d.rearrange("(k p) s -> p k s", p=128)
    wq_r = wq_d.rearrange("(k p) f -> p k f", p=128)
    wk_r = wk_d.rearrange("(k p) f -> p k f", p=128)
    wv_r = wv_d.rearrange("(k p) f -> p k f", p=128)
    wo_r = wo_d.rearrange("(k p) e -> p k e", p=128)
    w1_r = w1_d.rearrange("(k p) h -> p k h", p=128)    # [128, KB, HID]
    w3_r = w3_d.rearrange("(k p) h -> p k h", p=128)
    w2_r = w2_d.rearrange("(k p) e -> p k e", p=128)    # [128, MB, E]

    with tile.TileContext(nc) as tc, contextlib.ExitStack() as ctx:
        consts = ctx.enter_context(tc.tile_pool(name="consts", bufs=1))
        ones_b = consts.tile([128, 1], BF16)
        nc.vector.memset(ones_b[:], 1.0)
        # dual-fp8 LDWEIGHTS needs the pair stride %16 bytes: pad dim-2
        ones8_t = consts.tile([128, 2, 16], F8E4)
        nc.vector.memset(ones8_t[:], 1.0)
        ones8 = ones8_t[:, :, 0:1]
        eps_t = consts.tile([128, 1], F32)
        nc.vector.memset(eps_t[:], EPS)
        # exp bias ln(1/2): E = exp(score)/2 keeps e4m3 under its 240 max
        ebias = consts.tile([128, 1], F32)
        nc.vector.memset(ebias[:], -0.6931471805599453)
        normw_sb = consts.tile([128, 2 * EB], F32)
        nc.sync.dma_start(out=normw_sb[:], in_=normw_d[:])
        lam_sb = consts.tile([1, 1], F32)
        nc.sync.dma_start(out=lam_sb[:], in_=lam_d[:])

        dram = ctx.enter_context(
            tc.tile_pool(name="dram", bufs=1, space="DRAM"))
        rs_ins = [dram.tile([4, E // ECH, SQ], BF16, name=f"rsin{i}")
                  for i in range(ECH)]
        rs_outs = [dram.tile([E // ECH, SQ], BF16, name=f"rsout{i}")
                   for i in range(ECH)]

        for _rep in range(reps):
            rp = contextlib.ExitStack()
            xtp = rp.enter_context(tc.tile_pool(name="xtp", bufs=1))
            xt = xtp.tile([128, EB, S], F16)
            wop = rp.enter_context(tc.tile_pool(name="wop", bufs=1))
            wo_sb = wop.tile([128, FB, E], F8E4)
            w13p = rp.enter_context(tc.tile_pool(name="w13p", bufs=2))

            w13_slabs = {}

            def w13_slab(i):
                if i not in w13_slabs:
                    w1s = w13p.tile([128, KB, 256], F16, tag="w1s")
                    w3s = w13p.tile([128, KB, 256], F16, tag="w3s")
                    nc.sync.dma_start(
                        out=w1s[:], in_=w1_r[:, :, i * 256:(i + 1) * 256])
                    nc.sync.dma_start(
                        out=w3s[:], in_=w3_r[:, :, i * 256:(i + 1) * 256])
                    w13_slabs[i] = (w1s, w3s)
                return w13_slabs[i]

            w2_slabs = {}

            def w2_slab(i):
                if i not in w2_slabs:
                    w2s = w2p.tile([128, MB, 256], F16, tag="w2s")
                    nc.sync.dma_start(
                        out=w2s[:], in_=w2_r[:, :, i * 256:(i + 1) * 256])
                    w2_slabs[i] = w2s
                return w2_slabs[i]

            attn = contextlib.ExitStack()
            qkv = attn.enter_context(tc.tile_pool(name="qkv", bufs=1))
            qt = qkv.tile([128, FB, S], F16)
            kt = qkv.tile([128, FB, S], F16)
            vt = qkv.tile([128, TB, VD], F8E4)

            # ================= phase P: projections ======================
            # fp16 throughout (1 cyc/row, same speed as bf16, 10-bit
            # mantissa): matmul noise must stay ~bf16-grade here — fp8's
            # 3.6%/elem noise passes through GEMMs undamped.  Only the
            # softmax-averaged / attenuated paths (es, vt, un8@wo8) are
            # fp8.
            with contextlib.ExitStack() as pp:
                wfp = pp.enter_context(tc.tile_pool(name="wfp", bufs=2))
                prj = pp.enter_context(
                    tc.tile_pool(name="prj", bufs=3, space="PSUM"))

                first = True
                for wr, dst in ((wq_r, qt), (wk_r, kt)):
                    wf = wfp.tile([128, KB, HF], F16, tag="wfull")
                    if first:
                        for kc in range(0, KB, 4):
                            nc.sync.dma_start(out=xt[:, kc:kc + 4, :],
                                              in_=xt_r[:, kc:kc + 4, :])
                            nc.sync.dma_start(out=wf[:, kc:kc + 4, :],
                                              in_=wr[:, kc:kc + 4, :])
                        first = False
                    else:
                        for kc in range(0, KB, 4):
                            nc.sync.dma_start(out=wf[:, kc:kc + 4, :],
                                              in_=wr[:, kc:kc + 4, :])
                    for m in range(FB):
                        ps = prj.tile([128, S], F32, tag="prjps")
                        for k in range(KB):
                            for hf in range(2):
                                sl = slice(hf * 512, (hf + 1) * 512)
                                nc.tensor.matmul(
                                    ps[:, sl],
                                    wf[:, k, m * 128:(m + 1) * 128],
                                    xt[:, k, sl],
                                    start=(k == 0), stop=(k == KB - 1))
                        nc.scalar.activation(out=dst[:, m, :], in_=ps[:],
                                             func=AF.Copy, scale=1.0)

                wf = wfp.tile([128, KB, VD], F16, tag="wfull")
                nc.sync.dma_start(out=wf[:], in_=wv_r[:])
                for m in range(TB):
                    ps = prj.tile([128, VD], F32, tag="prjps")
                    for k in range(KB):
                        for hf in range(2):
                            sl = slice(hf * 512, (hf + 1) * 512)
                            nc.tensor.matmul(
                                ps[:, sl],
                                xt[:, k, m * 128:(m + 1) * 128],
                                wf[:, k, sl],
                                start=(k == 0), stop=(k == KB - 1))
                    nc.scalar.activation(out=vt[:, m, :], in_=ps[:],
                                         func=AF.Copy, scale=1.0)

            # ================= phase H: heads ============================
            hw_stk = contextlib.ExitStack()
            u8p = hw_stk.enter_context(tc.tile_pool(name="u8p", bufs=1))
            un8 = u8p.tile([128, FB, S], F8E4)
            with contextlib.ExitStack() as ph:
                un_pool = ph.enter_context(tc.tile_pool(name="un", bufs=1))
                un = un_pool.tile([128, FB, S], BF16)
                # Wo load + FFN slab prefetch into the idle H DMA window
                for kc in range(0, FB, 4):
                    nc.sync.dma_start(out=wo_sb[:, kc:kc + 4, :],
                                      in_=wo_r[:, kc:kc + 4, :])
                for i in range(2):
                    w13_slab(i)

                ep = ph.enter_context(tc.tile_pool(name="ep", bufs=2))
                sc_ps = ph.enter_context(
                    tc.tile_pool(name="scps", bufs=2, space="PSUM"))
                u_ps = ph.enter_context(
                    tc.tile_pool(name="ups", bufs=1, space="PSUM"))
                r_ps = ph.enter_context(
                    tc.tile_pool(name="rps", bufs=1, space="PSUM"))
                rows = ph.enter_context(tc.tile_pool(name="rows", bufs=1))
                ssb = ph.enter_context(tc.tile_pool(name="ssb", bufs=1))
                rstdp = ph.enter_context(tc.tile_pool(name="rstdp", bufs=2))
                bcp = ph.enter_context(tc.tile_pool(name="bcp", bufs=1))
                sqp = ph.enter_context(tc.tile_pool(name="sqp", bufs=2))
                cmb = ph.enter_context(tc.tile_pool(name="cmb", bufs=2))

                # per-(h,half) sum-of-squares, one free-dim-packed bf16
                # tile per head, all at partition 0 (engine reads/writes
                # at partition offsets are rejected or racy)
                ss_h = [ssb.tile([1, 2, 512], BF16, tag=f"ss{h}",
                                 name=f"ss{h}") for h in range(HG)]

                ev_q = deque()

                def ev_head(h, es):
                    """Yield emit-closures for rows/ratio/U/combine of
                    head h (es complete by the time these run)."""
                    for half in range(2):
                        sl = slice(half * 512, (half + 1) * 512)
                        # PE outputs must start at partition 0, and walrus
                        # rejects quadrant offsets: one bank per row-stat.
                        r0 = r_ps.tile([1, 512], F32, tag="r0", name="r0")
                        r1 = r_ps.tile([1, 512], F32, tag="r1", name="r1")
                        rj = (r0, r1)

                        def rows_mm(tb, j, rj=rj, sl=sl, es=es):
                            nc.tensor.matmul(
                                rj[j][:], ones8[:],
                                es[j][:, tb:tb + 2, sl],
                                start=(tb == 0), stop=(tb == TB - 2),
                                perf_mode=PM_DR)
                        for tb in range(0, TB, 2):
                            for j in range(2):
                                yield lambda tb=tb, j=j, f=rows_mm: f(tb, j)

                        bc_box = []

                        def ratio_ops(rj=rj, bc_box=bc_box):
                            recip2 = rows.tile([1, 512], F32, tag="rc2")
                            nc.vector.reciprocal(out=recip2[:],
                                                 in_=rj[1][:])
                            ratio = rows.tile([1, 512], F32, tag="ratio")
                            nc.vector.tensor_mul(out=ratio[:],
                                                 in0=rj[0][:],
                                                 in1=recip2[:])
                            nc.vector.tensor_scalar(
                                out=ratio[:], in0=ratio[:],
                                scalar1=lam_sb[0:1, :], scalar2=None,
                                op0=ALU.mult)
                            bc = bcp.tile([128, 512], F32, tag="bcr")
                            nc.gpsimd.partition_broadcast(bc[:], ratio[:])
                            bc_box.append(bc)
                        yield ratio_ops

                        ss_box = []

                        def ss_alloc(ss_box=ss_box):
                            # reuses r0's bank: ratio has consumed r0/r1
                            # by the time the first ss matmul lands
                            ssx = r_ps.tile([1, 512], F32, tag="r0",
                                            name="ssx")
                            ss_box.append(ssx)
                        yield ss_alloc

                        for c in range(2):
                            box = []

                            def u_alloc(box=box):
                                u0 = u_ps.tile([128, 512], F32, tag="u0",
                                               name="u0")
                                u1 = u_ps.tile([128, 512], F32, tag="u1",
                                               name="u1")
                                box.append((u0, u1))
                            yield u_alloc

                            def u_mm(tb, j, box=box, c=c, h=h, sl=sl, es=es):
                                nc.tensor.matmul(
                                    box[0][j][:],
                                    vt[:, tb:tb + 2,
                                       (2 * h + c) * 128:
                                       (2 * h + c + 1) * 128],
                                    es[j][:, tb:tb + 2, sl],
                                    start=(tb == 0), stop=(tb == TB - 2),
                                    perf_mode=PM_DR)
                            for tb in range(0, TB, 2):
                                for j in range(2):
                                    yield lambda tb=tb, j=j, f=u_mm: f(tb, j)

                            def combine(box=box, bc_box=bc_box, c=c, h=h,
                                        half=half, sl=sl, ss_box=ss_box):
                                u0, u1 = box[0]
                                t2 = cmb.tile([128, 512], BF16, tag="t2")
                                nc.vector.tensor_mul(out=t2[:], in0=u1[:],
                                                     in1=bc_box[0][:])
                                nc.vector.tensor_sub(
                                    out=un[:, 2 * h + c, sl],
                                    in0=u0[:], in1=t2[:])
                                sq = sqp.tile([128, 512], BF16, tag="sq")
                                nc.vector.tensor_mul(
                                    out=sq[:], in0=un[:, 2 * h + c, sl],
                                    in1=un[:, 2 * h + c, sl])
                                ssx = ss_box[0]
                                nc.tensor.matmul(
                                    ssx[:], ones_b[:], sq[:],
                                    start=(c == 0), stop=(c == 1))
                                if c == 1:
                                    nc.vector.tensor_copy(
                                        out=ss_h[h][:, half, :],
                                        in_=ssx[:])
                            yield combine

                def drain(n):
                    for _ in range(n):
                        if ev_q:
                            ev_q.popleft()()

                for h in range(HG):
                    es = [ep.tile([128, TB, S], F8E4, tag=f"e{j}",
                                  name=f"es{j}")
                          for j in range(2)]
                    for j in range(2):
                        for tb in range(TB):
                            ps = sc_ps.tile([128, S], F32, tag="sc")
                            for hf in range(2):
                                sl = slice(hf * 512, (hf + 1) * 512)
                                nc.tensor.matmul(
                                    ps[:, sl],
                                    kt[:, 2 * h + j,
                                       tb * 128:(tb + 1) * 128],
                                    qt[:, 2 * h + j, sl],
                                    start=True, stop=True)
                            nc.scalar.activation(
                                out=es[j][:, tb, :], in_=ps[:],
                                func=AF.Exp, scale=SCALE, bias=ebias[:])
                            drain(5)
                    ev_q.extend(ev_head(h, es))
                drain(len(ev_q))

                # batched subln rstd: all Lns, then all Exps — only two
                # LUT loads total (interleaving Ln/Exp would thrash the
                # natural_log <-> exp table sets)
                for h in range(HG):
                    nc.scalar.activation(out=ss_h[h][:], in_=ss_h[h][:],
                                         func=AF.Ln, scale=1.0 / 256.0,
                                         bias=eps_t[0:1, :])
                rstds = []
                for h in range(HG):
                    rstd = rstdp.tile([1, 2, 512], BF16, tag="rstd",
                                      name="rstd")
                    nc.scalar.activation(out=rstd[:], in_=ss_h[h][:],
                                         func=AF.Exp, scale=-0.5)
                    rstds.append(rstd)
                for h in range(HG):
                    for half in range(2):
                        sl = slice(half * 512, (half + 1) * 512)
                        bc = bcp.tile([128, 512], BF16, tag="bcn")
                        nc.gpsimd.partition_broadcast(
                            bc[:], rstds[h][:, half, :])
                        for c in range(2):
                            nc.vector.tensor_mul(
                                out=un8[:, 2 * h + c, sl],
                                in0=un[:, 2 * h + c, sl], in1=bc[:])

            # ====== phase W: Wo partials + residual -> ReduceScatter ======
            with contextlib.ExitStack() as pw:
                wo_ps = pw.enter_context(
                    tc.tile_pool(name="wops", bufs=2, space="PSUM"))
                evac = pw.enter_context(tc.tile_pool(name="evac", bufs=3))
                for m in range(EB):
                    ps = wo_ps.tile([128, S], F32, tag="wops")
                    for kp in range(FB // 2):
                        for hf in range(2):
                            sl = slice(hf * 512, (hf + 1) * 512)
                            nc.tensor.matmul(
                                ps[:, sl],
                                wo_sb[:, 2 * kp:2 * kp + 2,
                                      m * 128:(m + 1) * 128],
                                un8[:, 2 * kp:2 * kp + 2, sl],
                                start=(kp == 0), stop=(kp == FB // 2 - 1),
                                perf_mode=PM_DR)
                    # arow = ps/8 + x = 4*(partial + x/4): the RS payload
                    # carries a harmless 4x that attn-norm normalizes away
                    arow = evac.tile([128, S], BF16, tag="arow")
                    nc.vector.scalar_tensor_tensor(
                        out=arow[:], in0=ps[:], scalar=1.0 / 8.0,
                        in1=xt[:, m, :], op0=ALU.mult, op1=ALU.add)
                    ec, ml = divmod(m, EMB)
                    nc.sync.dma_start(
                        out=rs_ins[ec][:, ml * 128:(ml + 1) * 128, :]
                            .rearrange("q p s -> p q s"),
                        in_=arow[:].rearrange("p (q s) -> p q s", q=4))
                    if ml == EMB - 1:
                        nc.gpsimd.collective_compute(
                            "ReduceScatter", ALU.add,
                            replica_groups=GROUPS,
                            ins=[rs_ins[ec].opt()],
                            outs=[rs_outs[ec].opt()])
                # more FFN slab prefetch (runs during the RS)
                w13_slab(2)
            hw_stk.close()
            attn.close()   # frees qt/kt/vt
            w2p = rp.enter_context(tc.tile_pool(name="w2p", bufs=2))
            w2_slab(0)     # first w2 slab rides the RS window

            # ============= attn-norm on own s-quarter -> xb ===============
            pz = contextlib.ExitStack()
            xb_pool = pz.enter_context(tc.tile_pool(name="xb", bufs=1))
            xb = xb_pool.tile([128, EB, SQ], BF16)
            xb8 = xb_pool.tile([128, EB, SQ], F8E4)
            with contextlib.ExitStack() as pn:
                zp = pn.enter_context(tc.tile_pool(name="zp", bufs=1))
                zt = zp.tile([128, EB, SQ], BF16)
                sqp2 = pn.enter_context(tc.tile_pool(name="sqp2", bufs=3))
                rows2 = pn.enter_context(tc.tile_pool(name="rows2", bufs=2))
                ss_ps2 = pn.enter_context(
                    tc.tile_pool(name="ssps2", bufs=1, space="PSUM"))
                bcp2 = pn.enter_context(tc.tile_pool(name="bcp2", bufs=1))

                ssp = ss_ps2.tile([1, SQ], F32)
                for m in range(EB):
                    ec, ml = divmod(m, EMB)
                    nc.sync.dma_start(
                        out=zt[:, m, :],
                        in_=rs_outs[ec][ml * 128:(ml + 1) * 128, :])
                    sq = sqp2.tile([128, SQ], BF16, tag="sq2")
                    nc.vector.tensor_mul(out=sq[:], in0=zt[:, m, :],
                                         in1=zt[:, m, :])
                    nc.tensor.matmul(ssp[:], ones_b[:], sq[:],
                                     start=(m == 0), stop=(m == EB - 1))
                lnr2 = rows2.tile([1, SQ], F32, tag="lnr2")
                nc.scalar.activation(out=lnr2[:], in_=ssp[:], func=AF.Ln,
                                     scale=1.0 / E, bias=eps_t[0:1, :])
                rstd2 = rows2.tile([1, SQ], BF16, tag="rstd2")
                nc.scalar.activation(out=rstd2[:], in_=lnr2[:],
                                     func=AF.Exp, scale=-0.5)
                bc2 = bcp2.tile([128, SQ], BF16)
                nc.gpsimd.partition_broadcast(bc2[:], rstd2[:])
                for m in range(EB):
                    nc.vector.scalar_tensor_tensor(
                        out=xb[:, m, :], in0=zt[:, m, :],
                        scalar=normw_sb[:, m:m + 1], in1=bc2[:],
                        op0=ALU.mult, op1=ALU.mult)

            # ========== SwiGLU (own s-quarter, full hidden dim) ===========
            with contextlib.ExitStack() as pg:
                gp = pg.enter_context(tc.tile_pool(name="gp", bufs=1))
                g = gp.tile([128, MB, SQ], F16)
                h1p = pg.enter_context(
                    tc.tile_pool(name="h1p", bufs=2, space="PSUM"))
                h3p = pg.enter_context(
                    tc.tile_pool(name="h3p", bufs=2, space="PSUM"))
                sgp = pg.enter_context(tc.tile_pool(name="sgp", bufs=3))

                for sl_i in range(HSL):
                    w1s, w3s = w13_slab(sl_i)
                    for ml in range(2):
                        mh = sl_i * 2 + ml
                        p1t = h1p.tile([128, SQ], F32, tag="h1")
                        p3t = h3p.tile([128, SQ], F32, tag="h3")
                        for k in range(KB):
                            nc.tensor.matmul(
                                p1t[:],
                                w1s[:, k, ml * 128:(ml + 1) * 128],
                                xb[:, k, :],
                                start=(k == 0), stop=(k == KB - 1))
                            nc.tensor.matmul(
                                p3t[:],
                                w3s[:, k, ml * 128:(ml + 1) * 128],
                                xb[:, k, :],
                                start=(k == 0), stop=(k == KB - 1))
                        sg = sgp.tile([128, SQ], F16, tag="sg")
                        nc.scalar.activation(out=sg[:], in_=p1t[:],
                                             func=AF.Silu, scale=1.0)
                        nc.vector.tensor_mul(out=g[:, mh, :], in0=p3t[:],
                                             in1=sg[:])

                ff_ps = pg.enter_context(
                    tc.tile_pool(name="ffps", bufs=2, space="PSUM"))
                z2p = pg.enter_context(tc.tile_pool(name="z2p", bufs=1))
                z2 = z2p.tile([128, EB, SQ], BF16)
                ss_ps3 = pg.enter_context(
                    tc.tile_pool(name="ssps3", bufs=1, space="PSUM"))
                rows3 = pg.enter_context(tc.tile_pool(name="rows3", bufs=2))
                bcp3 = pg.enter_context(tc.tile_pool(name="bcp3", bufs=1))
                outp = pg.enter_context(tc.tile_pool(name="outp", bufs=3))

                ssp3 = ss_ps3.tile([1, SQ], F32)
                for sl_i in range(ESL):
                    w2s = w2_slab(sl_i)
                    for ml in range(2):
                        m = sl_i * 2 + ml
                        ps = ff_ps.tile([128, SQ], F32, tag="ffps")
                        for k in range(MB):
                            nc.tensor.matmul(
                                ps[:],
                                w2s[:, k, ml * 128:(ml + 1) * 128],
                                g[:, k, :],
                                start=(k == 0), stop=(k == MB - 1))
                        nc.vector.tensor_add(out=z2[:, m, :], in0=ps[:],
                                             in1=xb[:, m, :])
                        sq3 = outp.tile([128, SQ], BF16, tag="sq3")
                        nc.vector.tensor_mul(out=sq3[:], in0=z2[:, m, :],
                                             in1=z2[:, m, :])
                        nc.tensor.matmul(ssp3[:], ones_b[:], sq3[:],
                                         start=(m == 0), stop=(m == EB - 1))

                lnr3 = rows3.tile([1, SQ], F32, tag="lnr3")
                nc.scalar.activation(out=lnr3[:], in_=ssp3[:], func=AF.Ln,
                                     scale=1.0 / E, bias=eps_t[0:1, :])
                rstd3 = rows3.tile([1, SQ], BF16, tag="rstd3")
                nc.scalar.activation(out=rstd3[:], in_=lnr3[:],
                                     func=AF.Exp, scale=-0.5)
                bc3 = bcp3.tile([128, SQ], BF16)
                nc.gpsimd.partition_broadcast(bc3[:], rstd3[:])
                for m in range(EB):
                    ot = outp.tile([128, SQ], F32, tag="ot")
                    nc.vector.scalar_tensor_tensor(
                        out=ot[:], in0=z2[:, m, :],
                        scalar=normw_sb[:, EB + m:EB + m + 1], in1=bc3[:],
                        op0=ALU.mult, op1=ALU.mult)
                    nc.sync.dma_start(out=out_d[m * 128:(m + 1) * 128, :],
                                      in_=ot[:])

            pz.close()
            rp.close()

    nc.finalize()
    return nc


_NC_CACHE = None


def _get_nc():
    global _NC_CACHE
    if _NC_CACHE is None:
        _NC_CACHE = _build_nc()
    return _NC_CACHE


def _bf(x):
    return np.ascontiguousarray(np.asarray(x, np.float32)).astype(
        ml_dtypes.bfloat16)


def _f16(x):
    return np.ascontiguousarray(np.asarray(x, np.float32)).astype(
        np.float16)


def _f8(x):
    return np.ascontiguousarray(np.asarray(x, np.float32)).astype(
        ml_dtypes.float8_e4m3)


def make_in_maps(input_embeddings, Wq, Wk, Wv, Wo,
                 lam_q1, lam_k1, lam_q2, lam_k2, subln_w,
                 attn_norm_w, ff_norm_w, w1, w3, w2):
    x = np.asarray(input_embeddings, np.float32)
    Wo = np.asarray(Wo, np.float32)
    subln_w = np.asarray(subln_w, np.float32)

    lam = (np.exp(np.dot(np.asarray(lam_q1, np.float64),
                         np.asarray(lam_k1, np.float64)))
           - np.exp(np.dot(np.asarray(lam_q2, np.float64),
                           np.asarray(lam_k2, np.float64)))
           + LAMBDA_INIT)
    lam_arr = np.full((1, 1), lam, np.float32)

    wo_scaled = Wo * (np.tile(subln_w, H) * (1.0 - LAMBDA_INIT))[:, None]

    normw = np.zeros((128, 2 * EB), np.float32)
    normw[:, :EB] = np.asarray(attn_norm_w, np.float32).reshape(EB, 128).T
    normw[:, EB:] = np.asarray(ff_norm_w, np.float32).reshape(EB, 128).T

    Wq = np.asarray(Wq, np.float32)
    Wk = np.asarray(Wk, np.float32)
    Wv = np.asarray(Wv, np.float32)
    w1h = _f16(np.asarray(w1, np.float32))
    w3h = _f16(np.asarray(w3, np.float32))
    w2h = _f16(np.asarray(w2, np.float32))

    xts = [_f16(x[b].T) for b in range(B)]
    in_maps = []
    for c in range(N_CORES):
        b, g = c // 4, c % 4
        sl = slice(HF * g, HF * (g + 1))
        in_maps.append({
            "xt": xts[b],
            "wq": _f16(Wq[:, sl]),
            "wk": _f16(Wk[:, sl]),
            "wv": _f16(Wv[:, sl]),
            "wo": _f8(WS * wo_scaled[sl, :]),
            "w1": w1h,
            "w3": w3h,
            "w2": w2h,
            "normw": normw,
            "lam": lam_arr,
        })
    return in_maps


def assemble(results):
    out = np.empty((B, S, E), np.float32)
    for c in range(N_CORES):
        b, g = c // 4, c % 4
        out[b, SQ * g:SQ * (g + 1), :] = np.asarray(results[c]["out"]).T
    return out


def kernel(input_embeddings, attention_mask, Wq, Wk, Wv, Wo,
           lam_q1, lam_k1, lam_q2, lam_k2, subln_w,
           attn_norm_w, ff_norm_w, w1, w3, w2):
    in_maps = make_in_maps(input_embeddings, Wq, Wk, Wv, Wo,
                           lam_q1, lam_k1, lam_q2, lam_k2, subln_w,
                           attn_norm_w, ff_norm_w, w1, w3, w2)
    nc = _get_nc()
    res = run_bass_kernel_spmd(nc, in_maps, core_ids=list(range(N_CORES)))
    return assemble(res.results)
